# revision 1
# baseline (speedup 1.0000x reference)
"""MixerDiffAttention Trainium2 kernel.

Sharding: 8 cores = batch(2) x head-group(4).  Core (b, r) computes output
heads {2r, 2r+1} of batch b: q-heads {2r,2r+1,8+2r,8+2r+1}, k-heads {r, 4+r},
v-head r.  Inside: fused causal pipeline over 512-token chunks — project,
RMS+RoPE+scale in natural layout, PE-transpose to q^T/k^T, then attention as
S^T = K Q^T (max-free softmax, ones-column on V gives the denominator),
differential combine + RMS + gn.  Matmuls in float32r (TF32-like, full rate).
"""
import numpy as np
import concourse.bass as bass
import concourse.mybir as mybir
from concourse import bacc
from concourse.tile import TileContext
from concourse.bass_utils import run_bass_kernel_spmd

B, T, DM = 2, 2048, 2048
H, KVH, D = 16, 8, 128
TC = 512                  # token chunk (= q chunk)
NT = T // 128             # 16 token tiles
NCH = T // TC             # 4 chunks
NDM = DM // 128           # 16 contraction chunks
EPS = 1e-6
ROPE_BASE = 10000.0
LAMBDA_INIT = 0.8 - 0.6 * np.exp(-0.3 * 12)
F32 = mybir.dt.float32
MM_DT = mybir.dt.float32r
AF = mybir.ActivationFunctionType
ALU = mybir.AluOpType
ISQ = float(1.0 / np.sqrt(D))
MASK_NEG = -1e30


def _bc_mid(a, n):
    # [128, m] AP -> [128, n(bcast), m]
    return bass.AP(tensor=a.tensor, offset=a.offset, ap=[list(a.ap[0]), [0, n], list(a.ap[1])])


def _bc_last(a, n):
    # [128, m] AP -> [128, m, n(bcast)]
    return bass.AP(tensor=a.tensor, offset=a.offset, ap=[list(a.ap[0]), list(a.ap[1]), [0, n]])


def _build():
    nc = bacc.Bacc(None, target_bir_lowering=False)

    xT = nc.dram_tensor("xT", [DM, T], F32, kind="ExternalInput")
    wq_d = nc.dram_tensor("wq", [DM, 512], F32, kind="ExternalInput")
    wkv_d = nc.dram_tensor("wkv", [DM, 512], F32, kind="ExternalInput")
    scal_d = nc.dram_tensor("scal", [128, 4], F32, kind="ExternalInput")
    gn_d = nc.dram_tensor("gnw", [128, 2, 256], F32, kind="ExternalInput")
    neglam_d = nc.dram_tensor("neglam", [128, 1], F32, kind="ExternalInput")
    y_d = nc.dram_tensor("y", [T, 2, 256], F32, kind="ExternalOutput")

    # constant tables, laid out [128 partitions, NT tiles, ...] host-side
    pos = np.arange(T, dtype=np.float64)
    inv = ROPE_BASE ** (-np.arange(0, D, 2, dtype=np.float64) / D)  # (64,)
    ang = np.outer(pos, inv)
    cos_h = np.cos(ang).astype(np.float32).reshape(NT, 128, 64).transpose(1, 0, 2).copy()
    sin_h = np.sin(ang).astype(np.float32).reshape(NT, 128, 64).transpose(1, 0, 2).copy()
    logp_h = np.log(np.arange(1, T + 1, dtype=np.float64)).astype(np.float32)
    logp_h = logp_h.reshape(NT, 128, 1).transpose(1, 0, 2).copy()
    # wide causal mask: mask_j[p, f] = NEG if p + 128j > f ; mask_j = wide[:, f + (3-j)*128]
    pidx = np.arange(128).reshape(128, 1)
    g = np.arange(TC + 384).reshape(1, TC + 384)
    wide_h = np.where(pidx + 384 > g, np.float32(MASK_NEG), np.float32(0.0)).astype(np.float32)
    ident_h = np.eye(128, dtype=np.float32)

    cos_c = nc.inline_tensor(cos_h, "cos_c")
    sin_c = nc.inline_tensor(sin_h, "sin_c")
    logp_c = nc.inline_tensor(logp_h, "logp_c")
    wide_c = nc.inline_tensor(wide_h, "wide_c")
    ident_c = nc.inline_tensor(ident_h, "ident_c")

    with TileContext(nc) as tc:
        with (
            tc.tile_pool(name="wp", bufs=1) as wp,
            tc.tile_pool(name="cp", bufs=1) as cp,
            tc.tile_pool(name="xp", bufs=17) as xp,
            tc.tile_pool(name="kv", bufs=1) as kvp,
            tc.tile_pool(name="qt", bufs=2) as qtp,
            tc.tile_pool(name="wk", bufs=2) as wk,
            tc.tile_pool(name="pt", bufs=3) as ptp,
            tc.tile_pool(name="yo", bufs=4) as yop,
            tc.tile_pool(name="ps_p", bufs=2, space="PSUM") as ps_p,
            tc.tile_pool(name="ps_s", bufs=2, space="PSUM") as ps_s,
            tc.tile_pool(name="ps_o", bufs=4, space="PSUM") as ps_o,
        ):
            # ---- persistent loads ----
            wq_sb = wp.tile([128, NDM, 512], MM_DT, tag="wq")
            wkv_sb = wp.tile([128, NDM, 512], MM_DT, tag="wkv")
            nc.gpsimd.dma_start(out=wq_sb, in_=wq_d.ap().rearrange("(n p) m -> p n m", p=128))
            nc.gpsimd.dma_start(out=wkv_sb, in_=wkv_d.ap().rearrange("(n p) m -> p n m", p=128))

            cos_sb = cp.tile([128, NT, 64], F32, tag="cos")
            sin_sb = cp.tile([128, NT, 64], F32, tag="sin")
            logp_sb = cp.tile([128, NT, 1], F32, tag="logp")
            wide_sb = cp.tile([128, TC + 384], F32, tag="wide")
            ident_sb = cp.tile([128, 128], F32, tag="ident")
            scal_sb = cp.tile([128, 4], F32, tag="scal")
            gn_sb = cp.tile([128, 2, 256], F32, tag="gn")
            neglam_sb = cp.tile([128, 1], F32, tag="neglam")
            eps_sb = cp.tile([128, 1], F32, tag="eps")
            nc.sync.dma_start(out=cos_sb, in_=cos_c.ap())
            nc.sync.dma_start(out=sin_sb, in_=sin_c.ap())
            nc.sync.dma_start(out=logp_sb, in_=logp_c.ap())
            nc.sync.dma_start(out=wide_sb, in_=wide_c.ap())
            nc.sync.dma_start(out=ident_sb, in_=ident_c.ap())
            nc.sync.dma_start(out=scal_sb, in_=scal_d.ap())
            nc.sync.dma_start(out=gn_sb, in_=gn_d.ap())
            nc.sync.dma_start(out=neglam_sb, in_=neglam_d.ap())
            nc.vector.memset(eps_sb[:], EPS)

            # per-token-tile persistent K^T and V(+ones)
            kT_t = [kvp.tile([128, 2, 128], MM_DT, tag=f"kT{i}", name=f"kT{i}") for i in range(NT)]
            vA_t = [kvp.tile([128, 258], MM_DT, tag=f"vA{i}", name=f"vA{i}") for i in range(NT)]
            for i in range(NT):
                nc.vector.memset(vA_t[i][:, 256:258].bitcast(F32), 1.0)

            for c in range(NCH):
                # ---- load x^T chunk (cast to f32r) ----
                xts = []
                for dmi in range(NDM):
                    xt_t = xp.tile([128, TC], MM_DT, tag="xt")
                    nc.gpsimd.dma_start(
                        out=xt_t,
                        in_=xT.ap()[dmi * 128:(dmi + 1) * 128, c * TC:(c + 1) * TC],
                    )
                    xts.append(xt_t)

                qT_ch = qtp.tile([128, 4, TC], MM_DT, tag="qtc")

                for ti in range(4):
                    tt = c * 4 + ti
                    # ---- projections ----
                    q_ps = ps_p.tile([128, 512], F32, tag="pp")
                    kv_ps = ps_p.tile([128, 512], F32, tag="pp")
                    for dmi in range(NDM):
                        lhs = xts[dmi][:, ti * 128:(ti + 1) * 128]
                        nc.tensor.matmul(q_ps[:], lhs, wq_sb[:, dmi, :],
                                         start=(dmi == 0), stop=(dmi == NDM - 1))
                        nc.tensor.matmul(kv_ps[:], lhs, wkv_sb[:, dmi, :],
                                         start=(dmi == 0), stop=(dmi == NDM - 1))

                    # ---- q processing: copy, rms stats, scale, rope ----
                    q_sb = wk.tile([128, 512], F32, tag="q")
                    nc.scalar.copy(out=q_sb[:], in_=q_ps[:])
                    qr = wk.tile([128, 512], F32, tag="qr")       # scratch then rotated q
                    ssq = wk.tile([128, 4], F32, tag="ssq")
                    for h in range(4):
                        nc.scalar.activation(out=qr[:, h * 128:(h + 1) * 128],
                                             in_=q_ps[:, h * 128:(h + 1) * 128],
                                             func=AF.Square, accum_out=ssq[:, h:h + 1])
                    rsq = wk.tile([128, 4], F32, tag="rsq")
                    nc.scalar.activation(out=rsq[:], in_=ssq[:], func=AF.Sqrt,
                                         scale=1.0 / D, bias=eps_sb[:])
                    nc.vector.reciprocal(rsq[:], rsq[:])
                    nc.vector.tensor_scalar_mul(rsq[:], rsq[:], logp_sb[:, tt, :])
                    nc.vector.tensor_mul(rsq[:], rsq[:], scal_sb[:])
                    qv = q_sb[:].rearrange("p (h d) -> p h d", h=4)
                    nc.vector.tensor_mul(qv, qv, _bc_last(rsq[:], 128))
                    # rope
                    qrv = qr[:].rearrange("p (h d) -> p h d", h=4)
                    cos4 = _bc_mid(cos_sb[:, tt, :], 4)
                    sin4 = _bc_mid(sin_sb[:, tt, :], 4)
                    t1 = wk.tile([128, 4, 64], F32, tag="t1")
                    nc.vector.tensor_mul(qrv[:, :, 0:64], qv[:, :, 0:64], cos4)
                    nc.vector.tensor_mul(t1[:], qv[:, :, 64:128], sin4)
                    nc.vector.tensor_add(qrv[:, :, 0:64], qrv[:, :, 0:64], t1[:])
                    nc.vector.tensor_mul(qrv[:, :, 64:128], qv[:, :, 64:128], cos4)
                    nc.vector.tensor_mul(t1[:], qv[:, :, 0:64], sin4)
                    nc.vector.tensor_sub(qrv[:, :, 64:128], qrv[:, :, 64:128], t1[:])

                    # ---- k processing ----
                    k_sb = wk.tile([128, 256], F32, tag="k")
                    nc.scalar.copy(out=k_sb[:], in_=kv_ps[:, 0:256])
                    kr = wk.tile([128, 256], F32, tag="kr")
                    ssk = wk.tile([128, 2], F32, tag="ssk")
                    for h in range(2):
                        nc.scalar.activation(out=kr[:, h * 128:(h + 1) * 128],
                                             in_=kv_ps[:, h * 128:(h + 1) * 128],
                                             func=AF.Square, accum_out=ssk[:, h:h + 1])
                    rsk = wk.tile([128, 2], F32, tag="rsk")
                    nc.scalar.activation(out=rsk[:], in_=ssk[:], func=AF.Sqrt,
                                         scale=1.0 / D, bias=eps_sb[:])
                    nc.vector.reciprocal(rsk[:], rsk[:])
                    kv_ = k_sb[:].rearrange("p (h d) -> p h d", h=2)
                    nc.vector.tensor_mul(kv_, kv_, _bc_last(rsk[:], 128))
                    krv = kr[:].rearrange("p (h d) -> p h d", h=2)
                    cos2 = _bc_mid(cos_sb[:, tt, :], 2)
                    sin2 = _bc_mid(sin_sb[:, tt, :], 2)
                    t2 = wk.tile([128, 2, 64], F32, tag="t2")
                    nc.vector.tensor_mul(krv[:, :, 0:64], kv_[:, :, 0:64], cos2)
                    nc.vector.tensor_mul(t2[:], kv_[:, :, 64:128], sin2)
                    nc.vector.tensor_add(krv[:, :, 0:64], krv[:, :, 0:64], t2[:])
                    nc.vector.tensor_mul(krv[:, :, 64:128], kv_[:, :, 64:128], cos2)
                    nc.vector.tensor_mul(t2[:], kv_[:, :, 0:64], sin2)
                    nc.vector.tensor_sub(krv[:, :, 64:128], krv[:, :, 64:128], t2[:])

                    # ---- v (+ ones col already set) ----
                    nc.vector.tensor_copy(out=vA_t[tt][:, 0:256], in_=kv_ps[:, 256:512])

                    # ---- transposes ----
                    for h in range(4):
                        tp = ps_s.tile([128, 128], F32, tag="st")
                        nc.tensor.transpose(tp[:], qr[:, h * 128:(h + 1) * 128], ident_sb[:])
                        nc.any.tensor_copy(out=qT_ch[:, h, ti * 128:(ti + 1) * 128], in_=tp[:])
                    for h in range(2):
                        tp = ps_s.tile([128, 128], F32, tag="st")
                        nc.tensor.transpose(tp[:], kr[:, h * 128:(h + 1) * 128], ident_sb[:])
                        nc.any.tensor_copy(out=kT_t[tt][:, h, :], in_=tp[:])

                # ---- attention for q-chunk c ----
                for h in range(2):
                    y1 = wk.tile([128, 4, 256], F32, tag="y1")
                    for s in range(2):
                        o_t = [ps_o.tile([128, 258], F32, tag="o", name=f"o{_sq}") for _sq in range(4)]
                        for kt in range(4 * (c + 1)):
                            st = ps_s.tile([128, 512], F32, tag="st")
                            nc.tensor.matmul(st[:], kT_t[kt][:, s, :], qT_ch[:, 2 * s + h, :],
                                             start=True, stop=True)
                            j = kt - 4 * c
                            if j >= 0:
                                off = (3 - j) * 128
                                nc.vector.tensor_add(st[:], st[:], wide_sb[:, off:off + TC])
                            pt = ptp.tile([128, 512], MM_DT, tag="pt")
                            nc.scalar.activation(out=pt[:], in_=st[:], func=AF.Exp, scale=ISQ)
                            for sq in range(4):
                                qt_g = 4 * c + sq
                                if qt_g < kt:
                                    continue
                                nc.tensor.matmul(o_t[sq][:], pt[:, sq * 128:(sq + 1) * 128],
                                                 vA_t[kt][:], start=(kt == 0), stop=(kt == qt_g))
                        for sq in range(4):
                            ot = o_t[sq]
                            rec = wk.tile([128, 1], F32, tag="rec")
                            nc.vector.reciprocal(rec[:], ot[:, 256:257])
                            if s == 0:
                                nc.vector.tensor_scalar_mul(y1[:, sq, :], ot[:, 0:256], rec[:])
                            else:
                                nc.vector.tensor_mul(rec[:], rec[:], neglam_sb[:])
                                yv = wk.tile([128, 256], F32, tag="yv")
                                nc.vector.scalar_tensor_tensor(
                                    out=yv[:], in0=ot[:, 0:256], scalar=rec[:],
                                    in1=y1[:, sq, :], op0=ALU.mult, op1=ALU.add)
                                s2 = wk.tile([128, 1], F32, tag="s2")
                                sq2 = wk.tile([128, 256], F32, tag="sq2")
                                nc.scalar.activation(out=sq2[:], in_=yv[:], func=AF.Square,
                                                     accum_out=s2[:])
                                rs = wk.tile([128, 1], F32, tag="rs")
                                nc.scalar.activation(out=rs[:], in_=s2[:], func=AF.Sqrt,
                                                     scale=1.0 / 256, bias=eps_sb[:])
                                nc.vector.reciprocal(rs[:], rs[:])
                                yo = yop.tile([128, 256], F32, tag="yo")
                                nc.vector.scalar_tensor_tensor(
                                    out=yo[:], in0=yv[:], scalar=rs[:],
                                    in1=gn_sb[:, h, :], op0=ALU.mult, op1=ALU.mult)
                                qt_g = 4 * c + sq
                                nc.sync.dma_start(
                                    out=y_d.ap()[qt_g * 128:(qt_g + 1) * 128, h, :],
                                    in_=yo[:])
    nc.compile()
    return nc


_NC = None
_last_in_maps = None


def _get_nc():
    global _NC
    if _NC is None:
        _NC = _build()
    return _NC


def kernel(x, Wq, Wk, Wv, lambda_q1, lambda_k1, lambda_q2, lambda_k2,
           softmax_scaler, gn_weight):
    x = np.asarray(x, np.float32)
    Wq = np.asarray(Wq, np.float32)
    Wk = np.asarray(Wk, np.float32)
    Wv = np.asarray(Wv, np.float32)
    lam = float(np.exp(np.sum(np.float64(lambda_q1) * np.float64(lambda_k1)))
                - np.exp(np.sum(np.float64(lambda_q2) * np.float64(lambda_k2)))
                + LAMBDA_INIT)
    softmax_scaler = np.asarray(softmax_scaler, np.float32)
    gn_weight = np.asarray(gn_weight, np.float32)

    nc = _get_nc()
    in_maps = []
    for core in range(8):
        b, r = divmod(core, 4)
        qheads = [2 * r, 2 * r + 1, 8 + 2 * r, 8 + 2 * r + 1]
        wq_c = np.concatenate([Wq[:, hh * 128:(hh + 1) * 128] for hh in qheads], axis=1)
        wkv_c = np.concatenate([
            Wk[:, r * 128:(r + 1) * 128],
            Wk[:, (4 + r) * 128:(5 + r) * 128],
            Wv[:, r * 256:(r + 1) * 256],
        ], axis=1)
        in_maps.append({
            "xT": np.ascontiguousarray(x[b].T),
            "wq": np.ascontiguousarray(wq_c),
            "wkv": np.ascontiguousarray(wkv_c),
            "scal": np.ascontiguousarray(
                np.broadcast_to(softmax_scaler[qheads].reshape(1, 4), (128, 4))),
            "gnw": np.ascontiguousarray(
                np.broadcast_to(gn_weight[2 * r:2 * r + 2].reshape(1, 2, 256), (128, 2, 256))),
            "neglam": np.full((128, 1), -lam, np.float32),
        })
    global _last_in_maps
    _last_in_maps = in_maps
    res = run_bass_kernel_spmd(nc, in_maps, list(range(8)))
    out = np.empty((B, T, 8, 256), np.float32)
    for core in range(8):
        b, r = divmod(core, 4)
        out[b, :, 2 * r:2 * r + 2, :] = res.results[core]["y"]
    return out



# revision 15
# speedup vs baseline: 1.4121x; 1.4121x over previous
"""MixerDiffAttention Trainium2 kernel (v2).

Sharding: 8 cores = batch(2) x head-group(4).  Core (b, r) computes output
heads {2r, 2r+1} of batch b: q-heads {2r,2r+1,8+2r,8+2r+1}, k-heads {r, 4+r},
v-head r.

v2 design vs baseline:
 - Act engine runs ONLY Exp (no activation-table reloads).  RMS sums of
   squares via DVE mul+reduce, rsqrt via bit-trick + Newton on DVE, PSUM
   drains and epilogue on the Pool engine.
 - q/k pipeline in fp16 (host-cast inputs); scores matmul fp16, PV matmul
   bf16 (pt needs bf16 range for max-free exp).  DVE gets 2-byte fast modes.
 - Transposes via DMA xbar (16-bit) instead of PE matmul transposes.
 - Score matmul + exp sliced to skip fully-masked diagonal regions; only the
   diagonal 128x128 block gets a mask add.
 - Program order interleaves proj(c+2) into attn(c) so the PE never idles
   waiting for exp, keeping it at full p-state.
"""
import os
import numpy as np
import concourse.bass as bass
import concourse.mybir as mybir
from concourse import bacc
from concourse.tile import TileContext
from concourse.bass_utils import run_bass_kernel_spmd

B, T, DM = 2, 2048, 2048
H, KVH, D = 16, 8, 128
TC = 512                  # token chunk (= q chunk)
NT = T // 128             # 16 token tiles
NCH = T // TC             # 4 chunks
NDM = DM // 128           # 16 contraction chunks
EPS = 1e-6
ROPE_BASE = 10000.0
LAMBDA_INIT = 0.8 - 0.6 * np.exp(-0.3 * 12)
F32 = mybir.dt.float32
FP16 = mybir.dt.float16
BF16 = mybir.dt.bfloat16
I32 = mybir.dt.int32
AF = mybir.ActivationFunctionType
ALU = mybir.AluOpType
AX = mybir.AxisListType
ISQ = float(1.0 / np.sqrt(D))
MASK_NEG = -1e30
RSQRT_MAGIC = 0x5F3759DF


def _bc_mid(a, n):
    # [128, m] AP -> [128, n(bcast), m]
    return bass.AP(tensor=a.tensor, offset=a.offset, ap=[list(a.ap[0]), [0, n], list(a.ap[1])])


def _bc_last(a, n):
    # [128, m] AP -> [128, m, n(bcast)]
    return bass.AP(tensor=a.tensor, offset=a.offset, ap=[list(a.ap[0]), list(a.ap[1]), [0, n]])


def _build():
    nc = bacc.Bacc(None, target_bir_lowering=False)

    xT = nc.dram_tensor("xT", [DM, T], FP16, kind="ExternalInput")
    wq_d = nc.dram_tensor("wq", [DM, 512], FP16, kind="ExternalInput")
    wkv_d = nc.dram_tensor("wkv", [DM, 512], FP16, kind="ExternalInput")
    scal_d = nc.dram_tensor("scal", [128, 4], F32, kind="ExternalInput")
    gn_d = nc.dram_tensor("gnw", [128, 2, 256], F32, kind="ExternalInput")
    neglam_d = nc.dram_tensor("neglam", [128, 1], F32, kind="ExternalInput")
    y_d = nc.dram_tensor("y", [T, 2, 256], F32, kind="ExternalOutput")

    # constant tables, laid out [128 partitions, NT tiles, ...] host-side
    pos = np.arange(T, dtype=np.float64)
    inv = ROPE_BASE ** (-np.arange(0, D, 2, dtype=np.float64) / D)  # (64,)
    ang = np.outer(pos, inv)                                       # (T, 64)
    cos_t = np.cos(ang).reshape(NT, 128, 64)
    sin_t = np.sin(ang).reshape(NT, 128, 64)
    # full-width tables replicated over 4 heads: [128, NT, 4, 128]
    cosf = np.concatenate([cos_t, cos_t], axis=2)          # (NT,128,128)
    sinf = np.concatenate([sin_t, -sin_t], axis=2)
    cosf4 = np.broadcast_to(cosf[:, :, None, :], (NT, 128, 4, 128))
    sinf4 = np.broadcast_to(sinf[:, :, None, :], (NT, 128, 4, 128))
    cosf4_h = cosf4.transpose(1, 0, 2, 3).astype(np.float16).copy()
    sinf4_h = sinf4.transpose(1, 0, 2, 3).astype(np.float16).copy()
    logp_h = np.log(np.arange(1, T + 1, dtype=np.float64)).astype(np.float32)
    logp_h = logp_h.reshape(NT, 128, 1).transpose(1, 0, 2).copy()
    pidx = np.arange(128).reshape(128, 1)
    fidx = np.arange(128).reshape(1, 128)
    tri_h = np.where(pidx > fidx, np.float32(MASK_NEG), np.float32(0.0))

    cos_c = nc.inline_tensor(cosf4_h, "cos_c")
    sin_c = nc.inline_tensor(sinf4_h, "sin_c")
    logp_c = nc.inline_tensor(logp_h, "logp_c")
    tri_c = nc.inline_tensor(tri_h, "tri_c")

    with TileContext(nc) as tc:
        with (
            tc.tile_pool(name="wp", bufs=1) as wp,
            tc.tile_pool(name="cp", bufs=1) as cp,
            tc.tile_pool(name="xp", bufs=34) as xp,
            tc.tile_pool(name="kv", bufs=1) as kvp,
            tc.tile_pool(name="qt", bufs=2) as qtp,
            tc.tile_pool(name="wk", bufs=2) as wk,
            tc.tile_pool(name="qh", bufs=6) as qhp,
            tc.tile_pool(name="qr", bufs=4) as qrp,
            tc.tile_pool(name="pt", bufs=30) as ptp,
            tc.tile_pool(name="yv", bufs=5) as yvp,
            tc.tile_pool(name="yo", bufs=4) as yop,
            tc.tile_pool(name="ps_p", bufs=2, space="PSUM") as ps_p,
            tc.tile_pool(name="ps_s", bufs=4, space="PSUM") as ps_s,
            tc.tile_pool(name="ps_o", bufs=2, space="PSUM") as ps_o,
        ):
            # ---- persistent loads ----
            wq_sb = wp.tile([128, NDM, 512], FP16, tag="wq")
            wkv_sb = wp.tile([128, NDM, 512], FP16, tag="wkv")
            nc.sync.dma_start(out=wq_sb, in_=wq_d.ap().rearrange("(n p) m -> p n m", p=128))
            nc.sync.dma_start(out=wkv_sb, in_=wkv_d.ap().rearrange("(n p) m -> p n m", p=128))

            cos_sb = cp.tile([128, NT, 4, 128], FP16, tag="cos")
            sin_sb = cp.tile([128, NT, 4, 128], FP16, tag="sin")
            logp_sb = cp.tile([128, NT, 1], F32, tag="logp")
            tri_sb = cp.tile([128, 128], F32, tag="tri")
            scal_sb = cp.tile([128, 4], F32, tag="scal")
            gn_sb = cp.tile([128, 2, 256], F32, tag="gn")
            neglam_sb = cp.tile([128, 1], F32, tag="neglam")
            magic_sb = cp.tile([128, 24], I32, tag="magic")
            nc.sync.dma_start(out=cos_sb, in_=cos_c.ap())
            nc.sync.dma_start(out=sin_sb, in_=sin_c.ap())
            nc.sync.dma_start(out=logp_sb, in_=logp_c.ap())
            nc.sync.dma_start(out=tri_sb, in_=tri_c.ap())
            nc.sync.dma_start(out=scal_sb, in_=scal_d.ap())
            nc.sync.dma_start(out=gn_sb, in_=gn_d.ap())
            nc.sync.dma_start(out=neglam_sb, in_=neglam_d.ap())
            nc.vector.memset(magic_sb[:], RSQRT_MAGIC)

            # per-token-tile persistent K^T and V(+ones)
            kT_t = [kvp.tile([128, 2, 128], FP16, tag=f"kT{i}", name=f"kT{i}") for i in range(NT)]
            vA_t = [kvp.tile([128, 258], BF16, tag=f"vA{i}", name=f"vA{i}") for i in range(NT)]
            for i in range(NT):
                nc.gpsimd.memset(vA_t[i][:, 256:258], 1.0)

            xts = {}

            def emit_x_load(c):
                tiles = []
                for dmi in range(NDM):
                    xt_t = xp.tile([128, TC], FP16, tag="xt")
                    nc.sync.dma_start(
                        out=xt_t,
                        in_=xT.ap()[dmi * 128:(dmi + 1) * 128, c * TC:(c + 1) * TC],
                    )
                    tiles.append(xt_t)
                xts[c] = tiles

            ssq_ch = {}   # [128, 4, 6] f32 per chunk
            qh_ch = {}    # list of (q_h, k_h) per chunk
            rs_ch = {}    # (rsq_h [128,4,4] fp16, rsk_h [128,4,2] fp16)
            qT_ch = {}    # [128, 4, TC] fp16 per chunk (rotating pool bufs=2)

            def proj_tile(c, ti):
                tt = c * 4 + ti
                q_ps = ps_p.tile([128, 512], F32, tag="pp")
                kv_ps = ps_p.tile([128, 512], F32, tag="pp")
                for dmi in range(NDM):
                    lhs = xts[c][dmi][:, ti * 128:(ti + 1) * 128]
                    nc.tensor.matmul(q_ps[:], lhs, wq_sb[:, dmi, :],
                                     start=(dmi == 0), stop=(dmi == NDM - 1))
                for dmi in range(NDM):
                    lhs = xts[c][dmi][:, ti * 128:(ti + 1) * 128]
                    nc.tensor.matmul(kv_ps[:], lhs, wkv_sb[:, dmi, :],
                                     start=(dmi == 0), stop=(dmi == NDM - 1))
                # drain PSUM fast on Pool; squares+reduce on DVE
                q_h = qhp.tile([128, 512], FP16, tag="qh")
                k_h = qhp.tile([128, 256], FP16, tag="kh")
                nc.vector.tensor_copy(out=q_h[:], in_=q_ps[:])
                nc.vector.tensor_copy(out=k_h[:], in_=kv_ps[:, 0:256])
                nc.vector.tensor_copy(out=vA_t[tt][:, 0:256], in_=kv_ps[:, 256:512])
                if ti == 0:
                    ssq_ch[c] = wk.tile([128, 4, 6], F32, tag="ssq", name=f"ssq{c}")
                    qh_ch[c] = []
                qh_ch[c].append((q_h, k_h))
                sqv = wk.tile([128, 4, 128], FP16, tag="sqv")
                nc.vector.tensor_mul(sqv[:], q_h[:].rearrange("p (h d) -> p h d", h=4),
                                     q_h[:].rearrange("p (h d) -> p h d", h=4))
                nc.vector.tensor_reduce(ssq_ch[c][:, ti, 0:4], sqv[:], axis=AX.X, op=ALU.add)
                skv = wk.tile([128, 2, 128], FP16, tag="skv")
                nc.vector.tensor_mul(skv[:], k_h[:].rearrange("p (h d) -> p h d", h=2),
                                     k_h[:].rearrange("p (h d) -> p h d", h=2))
                nc.vector.tensor_reduce(ssq_ch[c][:, ti, 4:6], skv[:], axis=AX.X, op=ALU.add)

            def newton_rsqrt(ms_ap, n, tag):
                # in: ms_ap [128, n] f32 (mean-square + eps already applied)
                # returns [128, n] f32 tile of rsqrt(ms)
                sh = wk.tile([128, n], I32, tag=tag + "sh")
                nc.vector.tensor_scalar(out=sh[:], in0=ms_ap.bitcast(I32), scalar1=1,
                                        scalar2=None, op0=ALU.logical_shift_right)
                y0 = wk.tile([128, n], F32, tag=tag + "y0")
                nc.vector.tensor_tensor(out=y0[:].bitcast(I32), in0=magic_sb[:, 0:n],
                                        in1=sh[:], op=ALU.subtract)
                a = wk.tile([128, n], F32, tag=tag + "a")
                for _ in range(2):
                    nc.vector.tensor_tensor(out=a[:], in0=ms_ap, in1=y0[:], op=ALU.mult)
                    nc.vector.tensor_tensor(out=a[:], in0=a[:], in1=y0[:], op=ALU.mult)
                    nc.vector.tensor_scalar(out=a[:], in0=a[:], scalar1=-0.5, scalar2=1.5,
                                            op0=ALU.mult, op1=ALU.add)
                    nc.vector.tensor_tensor(out=y0[:], in0=y0[:], in1=a[:], op=ALU.mult)
                return y0

            def newton_chunk(c):
                ms = wk.tile([128, 24], F32, tag="ms")
                nc.vector.tensor_scalar(out=ms[:], in0=ssq_ch[c][:].rearrange("p a b -> p (a b)"),
                                        scalar1=1.0 / D, scalar2=EPS, op0=ALU.mult, op1=ALU.add)
                rs = newton_rsqrt(ms[:], 24, "nq")     # [128, 24] = [128, 4t, 6]
                rsv = rs[:].rearrange("p (t k) -> p t k", t=4)
                # q scale: rs * log(pos) * softmax_scaler
                rsq = wk.tile([128, 4, 4], F32, tag="rsq")
                nc.vector.tensor_mul(rsq[:], rsv[:, :, 0:4],
                                     _bc_last(logp_sb[:, 4 * c:4 * c + 4, 0], 4))
                nc.vector.tensor_mul(rsq[:], rsq[:], _bc_mid(scal_sb[:], 4))
                rsq_h = wk.tile([128, 4, 4], FP16, tag="rsqh")
                nc.vector.tensor_copy(out=rsq_h[:], in_=rsq[:])
                rsk_h = wk.tile([128, 4, 2], FP16, tag="rskh")
                nc.vector.tensor_copy(out=rsk_h[:], in_=rsv[:, :, 4:6])
                rs_ch[c] = (rsq_h, rsk_h)

            def rope_tile(c, ti):
                tt = c * 4 + ti
                q_h, k_h = qh_ch[c][ti]
                rsq_h, rsk_h = rs_ch[c]
                qs = wk.tile([128, 4, 128], FP16, tag="qs")
                nc.vector.tensor_mul(qs[:], q_h[:].rearrange("p (h d) -> p h d", h=4),
                                     _bc_last(rsq_h[:, ti, :], 128))
                qc = wk.tile([128, 4, 128], FP16, tag="qc")
                nc.vector.tensor_mul(qc[:], qs[:], cos_sb[:, tt])
                tq = wk.tile([128, 4, 128], FP16, tag="tq")
                nc.vector.tensor_mul(tq[:, :, 0:64], qs[:, :, 64:128], sin_sb[:, tt, :, 0:64])
                nc.vector.tensor_mul(tq[:, :, 64:128], qs[:, :, 0:64], sin_sb[:, tt, :, 64:128])
                qr = qrp.tile([128, 4, 128], FP16, tag="qr")
                nc.vector.tensor_add(qr[:], qc[:], tq[:])
                if ti == 0:
                    qT_ch[c] = qtp.tile([128, 4, TC], FP16, tag="qtc", name=f"qtc{c}")
                for h in range(4):
                    nc.sync.dma_start_transpose(
                        out=qT_ch[c][:, h, ti * 128:(ti + 1) * 128], in_=qr[:, h, :])
                ks = wk.tile([128, 2, 128], FP16, tag="ks")
                nc.gpsimd.tensor_mul(ks[:], k_h[:].rearrange("p (h d) -> p h d", h=2),
                                     _bc_last(rsk_h[:, ti, :], 128))
                kc = wk.tile([128, 2, 128], FP16, tag="kc")
                nc.gpsimd.tensor_mul(kc[:], ks[:], cos_sb[:, tt, 0:2])
                tk = wk.tile([128, 2, 128], FP16, tag="tk")
                nc.gpsimd.tensor_mul(tk[:, :, 0:64], ks[:, :, 64:128], sin_sb[:, tt, 0:2, 0:64])
                nc.gpsimd.tensor_mul(tk[:, :, 64:128], ks[:, :, 0:64], sin_sb[:, tt, 0:2, 64:128])
                kr = qrp.tile([128, 2, 128], FP16, tag="kr")
                nc.gpsimd.tensor_add(kr[:], kc[:], tk[:])
                for s in range(2):
                    nc.sync.dma_start_transpose(out=kT_t[tt][:, s, :], in_=kr[:, s, :])

            def qkv_chunk(c):
                for ti in range(4):
                    proj_tile(c, ti)
                newton_chunk(c)
                for ti in range(4):
                    rope_tile(c, ti)

            y1_ch = {}

            def attn_scores(c, h, s):
                pts = []
                for kt in range(4 * (c + 1)):
                    j = kt - 4 * c
                    qoff = max(j, 0) * 128
                    st = ps_s.tile([128, 512], F32, tag="st")
                    nc.tensor.matmul(st[:, qoff:512], kT_t[kt][:, s, :],
                                     qT_ch[c][:, 2 * s + h, qoff:512], start=True, stop=True)
                    if j >= 0:
                        nc.vector.tensor_add(st[:, qoff:qoff + 128], st[:, qoff:qoff + 128],
                                             tri_sb[:])
                    pt = ptp.tile([128, 512], BF16, tag="pt")
                    nc.scalar.activation(out=pt[:, qoff:512], in_=st[:, qoff:512],
                                         func=AF.Exp, scale=ISQ)
                    pts.append(pt)
                return pts

            def attn_pv(c, h, s, pts):
                if s == 0:
                    y1_ch[(c, h)] = wk.tile([128, 4, 256], F32, tag="y1", name=f"y1_{c}_{h}")
                y1 = y1_ch[(c, h)]
                yvs = []
                s2 = None
                if s == 1:
                    s2 = wk.tile([128, 4], F32, tag="s2")
                for sq in range(4):
                    qt_g = 4 * c + sq
                    o = ps_o.tile([128, 258], F32, tag="o")
                    for kt in range(qt_g + 1):
                        nc.tensor.matmul(o[:], pts[kt][:, sq * 128:(sq + 1) * 128],
                                         vA_t[kt][:], start=(kt == 0), stop=(kt == qt_g))
                    rec = wk.tile([128, 1], F32, tag="rec")
                    nc.vector.reciprocal(rec[:], o[:, 256:257])
                    if s == 0:
                        nc.vector.tensor_scalar_mul(y1[:, sq, :], o[:, 0:256], rec[:])
                    else:
                        nc.vector.tensor_scalar_mul(rec[:], rec[:], neglam_sb[:])
                        yv = yvp.tile([128, 256], F32, tag="yv")
                        nc.vector.scalar_tensor_tensor(
                            out=yv[:], in0=o[:, 0:256], scalar=rec[:],
                            in1=y1[:, sq, :], op0=ALU.mult, op1=ALU.add)
                        sq2 = wk.tile([128, 256], F32, tag="sq2")
                        nc.vector.tensor_mul(sq2[:], yv[:], yv[:])
                        nc.vector.tensor_reduce(s2[:, sq:sq + 1], sq2[:], axis=AX.X,
                                                op=ALU.add)
                        yvs.append(yv)
                if s == 1:
                    ms2 = wk.tile([128, 4], F32, tag="ms2")
                    nc.vector.tensor_scalar(out=ms2[:], in0=s2[:], scalar1=1.0 / 256,
                                            scalar2=EPS, op0=ALU.mult, op1=ALU.add)
                    rsy = newton_rsqrt(ms2[:], 4, "ne")
                    for sq in range(4):
                        qt_g = 4 * c + sq
                        yo = yop.tile([128, 256], F32, tag="yo")
                        nc.gpsimd.tensor_mul(yo[:], yvs[sq][:], gn_sb[:, h, :])
                        nc.gpsimd.tensor_mul(yo[:], yo[:], _bc_last(rsy[:, sq:sq + 1], 256))
                        nc.sync.dma_start(
                            out=y_d.ap()[qt_g * 128:(qt_g + 1) * 128, h, :], in_=yo[:])

            # ---------------- schedule ----------------
            HS = [(0, 0), (0, 1), (1, 0), (1, 1)]
            KVAR = int(os.environ.get("KVAR", "0"))
            if KVAR == 1:
                # qkv only: dump qT chunks into y for debugging
                emit_x_load(0)
                emit_x_load(1)
                for c in range(NCH):
                    if c + 2 < NCH:
                        emit_x_load(c + 2)
                    qkv_chunk(c)
                    nc.gpsimd.dma_start(
                        out=y_d.ap()[c * TC:(c + 1) * TC].rearrange("a b c -> a (b c)"),
                        in_=qT_ch[c][:].rearrange("p h t -> p (h t)"))
            elif KVAR == 0:
                emit_x_load(0)
                emit_x_load(1)
                for c in range(NCH):
                    if c + 2 < NCH:
                        emit_x_load(c + 2)
                    qkv_chunk(c)
                    for (h, s) in HS:
                        pts = attn_scores(c, h, s)
                        attn_pv(c, h, s, pts)
            elif KVAR == 3:
                # qkv + scores/exp only; dump last pt of each (c,h,s)
                emit_x_load(0)
                emit_x_load(1)
                for c in range(NCH):
                    if c + 2 < NCH:
                        emit_x_load(c + 2)
                    qkv_chunk(c)
                    for (h, s) in HS:
                        pts = attn_scores(c, h, s)
                        nc.gpsimd.dma_start(
                            out=y_d.ap()[c * TC:(c + 1) * TC, h, 0:128],
                            in_=pts[s][:, 0:512])
            elif KVAR == 2:
                emit_x_load(0)
                emit_x_load(1)
                qkv_chunk(0)
                emit_x_load(2)
                qkv_chunk(1)
                for c in (0, 1):
                    if c == 1:
                        emit_x_load(3)
                    cn = c + 2
                    for i, (h, s) in enumerate(HS):
                        pts = attn_scores(c, h, s)
                        proj_tile(cn, i)
                        attn_pv(c, h, s, pts)
                    newton_chunk(cn)
                    for ti in range(4):
                        rope_tile(cn, ti)
                for i, (h, s) in enumerate(HS):
                    pts2 = attn_scores(2, h, s)
                    pts3 = attn_scores(3, h, s)
                    attn_pv(2, h, s, pts2)
                    attn_pv(3, h, s, pts3)
    nc.compile()
    return nc


_NC = None
_last_in_maps = None


def _get_nc():
    global _NC
    if _NC is None:
        _NC = _build()
    return _NC


def kernel(x, Wq, Wk, Wv, lambda_q1, lambda_k1, lambda_q2, lambda_k2,
           softmax_scaler, gn_weight):
    x = np.asarray(x, np.float32)
    Wq = np.asarray(Wq, np.float32)
    Wk = np.asarray(Wk, np.float32)
    Wv = np.asarray(Wv, np.float32)
    lam = float(np.exp(np.sum(np.float64(lambda_q1) * np.float64(lambda_k1)))
                - np.exp(np.sum(np.float64(lambda_q2) * np.float64(lambda_k2)))
                + LAMBDA_INIT)
    softmax_scaler = np.asarray(softmax_scaler, np.float32)
    gn_weight = np.asarray(gn_weight, np.float32)

    nc = _get_nc()
    in_maps = []
    for core in range(8):
        b, r = divmod(core, 4)
        qheads = [2 * r, 2 * r + 1, 8 + 2 * r, 8 + 2 * r + 1]
        wq_c = np.concatenate([Wq[:, hh * 128:(hh + 1) * 128] for hh in qheads], axis=1)
        wkv_c = np.concatenate([
            Wk[:, r * 128:(r + 1) * 128],
            Wk[:, (4 + r) * 128:(5 + r) * 128],
            Wv[:, r * 256:(r + 1) * 256],
        ], axis=1)
        in_maps.append({
            "xT": np.ascontiguousarray(x[b].T).astype(np.float16),
            "wq": np.ascontiguousarray(wq_c).astype(np.float16),
            "wkv": np.ascontiguousarray(wkv_c).astype(np.float16),
            "scal": np.ascontiguousarray(
                np.broadcast_to(softmax_scaler[qheads].reshape(1, 4), (128, 4))).astype(np.float32),
            "gnw": np.ascontiguousarray(
                np.broadcast_to(gn_weight[2 * r:2 * r + 2].reshape(1, 2, 256), (128, 2, 256))).astype(np.float32),
            "neglam": np.full((128, 1), -lam, np.float32),
        })
    global _last_in_maps
    _last_in_maps = in_maps
    res = run_bass_kernel_spmd(nc, in_maps, list(range(8)))
    out = np.empty((B, T, 8, 256), np.float32)
    for core in range(8):
        b, r = divmod(core, 4)
        out[b, :, 2 * r:2 * r + 2, :] = res.results[core]["y"]
    return out


# revision 16
# speedup vs baseline: 1.4313x; 1.0136x over previous
"""MixerDiffAttention Trainium2 kernel (v3).

Sharding: 8 cores = batch(2) x head-group(4).  Core (b, r) computes output
heads {2r, 2r+1} of batch b: q-heads {2r,2r+1,8+2r,8+2r+1}, k-heads {r, 4+r},
v-head r.

Design:
 - Act engine runs ONLY Exp (zero activation-table reloads).
 - fp16 q/k pipeline (host-cast inputs), bf16 pt/vA for PV (bf16 range
   needed for max-free exp).  Scores fp16 x fp16 -> f32 PSUM.
 - Transposes via batched DMA xbar: one [128,512]->[128,4,128] per tile for
   q (sync queue) and one [128,256]->[128,2,128] for k (scalar queue).
 - Causal diag mask pre-loaded into PSUM by an extra matmul in the same
   accumulation group (no vector mask work).
 - RMS rsqrt via bit-trick + Newton on DVE; sums of squares via DVE
   mul+reduce; PSUM drains on DVE; SBUF-only elementwise (rope-k, final
   gn scale) on Pool.
 - Schedule: proj(c+2) + rope tiles interleaved into attn(c) segments so
   the PE and Act never wait on bursts.
"""
import os
import numpy as np
import concourse.bass as bass
import concourse.mybir as mybir
from concourse import bacc
from concourse.tile import TileContext
from concourse.bass_utils import run_bass_kernel_spmd

B, T, DM = 2, 2048, 2048
H, KVH, D = 16, 8, 128
TC = 512                  # token chunk (= q chunk)
NT = T // 128             # 16 token tiles
NCH = T // TC             # 4 chunks
NDM = DM // 128           # 16 contraction chunks
EPS = 1e-6
ROPE_BASE = 10000.0
LAMBDA_INIT = 0.8 - 0.6 * np.exp(-0.3 * 12)
F32 = mybir.dt.float32
FP16 = mybir.dt.float16
BF16 = mybir.dt.bfloat16
I32 = mybir.dt.int32
AF = mybir.ActivationFunctionType
ALU = mybir.AluOpType
AX = mybir.AxisListType
ISQ = float(1.0 / np.sqrt(D))
MASK_NEG = -60000.0       # fp16-safe; exp(ISQ*(s+MASK_NEG)) == 0 in f32
RSQRT_MAGIC = 0x5F3759DF


def _bc_mid(a, n):
    # [128, m] AP -> [128, n(bcast), m]
    return bass.AP(tensor=a.tensor, offset=a.offset, ap=[list(a.ap[0]), [0, n], list(a.ap[1])])


def _bc_last(a, n):
    # [128, m] AP -> [128, m, n(bcast)]
    return bass.AP(tensor=a.tensor, offset=a.offset, ap=[list(a.ap[0]), list(a.ap[1]), [0, n]])


def _build():
    nc = bacc.Bacc(None, target_bir_lowering=False)

    xT = nc.dram_tensor("xT", [DM, T], FP16, kind="ExternalInput")
    wq_d = nc.dram_tensor("wq", [DM, 512], FP16, kind="ExternalInput")
    wkv_d = nc.dram_tensor("wkv", [DM, 512], FP16, kind="ExternalInput")
    scal_d = nc.dram_tensor("scal", [128, 4], F32, kind="ExternalInput")
    gn_d = nc.dram_tensor("gnw", [128, 2, 256], F32, kind="ExternalInput")
    neglam_d = nc.dram_tensor("neglam", [128, 1], F32, kind="ExternalInput")
    y_d = nc.dram_tensor("y", [T, 2, 256], F32, kind="ExternalOutput")

    # constant tables, laid out [128 partitions, NT tiles, ...] host-side
    pos = np.arange(T, dtype=np.float64)
    inv = ROPE_BASE ** (-np.arange(0, D, 2, dtype=np.float64) / D)  # (64,)
    ang = np.outer(pos, inv)                                       # (T, 64)
    cos_t = np.cos(ang).reshape(NT, 128, 64)
    sin_t = np.sin(ang).reshape(NT, 128, 64)
    cosf = np.concatenate([cos_t, cos_t], axis=2)          # (NT,128,128)
    sinf = np.concatenate([sin_t, -sin_t], axis=2)
    cosf4 = np.broadcast_to(cosf[:, :, None, :], (NT, 128, 4, 128))
    sinf4 = np.broadcast_to(sinf[:, :, None, :], (NT, 128, 4, 128))
    cosf4_h = cosf4.transpose(1, 0, 2, 3).astype(np.float16).copy()
    sinf4_h = sinf4.transpose(1, 0, 2, 3).astype(np.float16).copy()
    logp_h = np.log(np.arange(1, T + 1, dtype=np.float64)).astype(np.float32)
    logp_h = logp_h.reshape(NT, 128, 1).transpose(1, 0, 2).copy()
    pidx = np.arange(128).reshape(128, 1)
    fidx = np.arange(512).reshape(1, 512)
    triw_h = np.where((pidx > fidx) & (fidx < 128), np.float16(MASK_NEG),
                      np.float16(0.0)).astype(np.float16)       # [128, 512]
    ident_h = np.eye(128, dtype=np.float16)

    cos_c = nc.inline_tensor(cosf4_h, "cos_c")
    sin_c = nc.inline_tensor(sinf4_h, "sin_c")
    logp_c = nc.inline_tensor(logp_h, "logp_c")
    triw_c = nc.inline_tensor(triw_h, "triw_c")
    ident_c = nc.inline_tensor(ident_h, "ident_c")

    with TileContext(nc) as tc:
        with (
            tc.tile_pool(name="wp", bufs=1) as wp,
            tc.tile_pool(name="cp", bufs=1) as cp,
            tc.tile_pool(name="xp", bufs=34) as xp,
            tc.tile_pool(name="kv", bufs=1) as kvp,
            tc.tile_pool(name="qt", bufs=2) as qtp,
            tc.tile_pool(name="wk", bufs=2) as wk,
            tc.tile_pool(name="qh", bufs=6) as qhp,
            tc.tile_pool(name="qr", bufs=4) as qrp,
            tc.tile_pool(name="pt", bufs=18) as ptp,
            tc.tile_pool(name="yv", bufs=5) as yvp,
            tc.tile_pool(name="yo", bufs=4) as yop,
            tc.tile_pool(name="ps_p", bufs=2, space="PSUM") as ps_p,
            tc.tile_pool(name="ps_s", bufs=4, space="PSUM") as ps_s,
            tc.tile_pool(name="ps_o", bufs=2, space="PSUM") as ps_o,
        ):
            # ---- persistent loads (spread across queues; wq + x(0) first) ----
            wq_sb = wp.tile([128, NDM, 512], FP16, tag="wq")
            wkv_sb = wp.tile([128, NDM, 512], FP16, tag="wkv")
            nc.sync.dma_start(out=wq_sb, in_=wq_d.ap().rearrange("(n p) m -> p n m", p=128))

            xts = {}

            def emit_x_load(c):
                tiles = []
                for dmi in range(NDM):
                    xt_t = xp.tile([128, TC], FP16, tag="xt")
                    nc.gpsimd.dma_start(
                        out=xt_t,
                        in_=xT.ap()[dmi * 128:(dmi + 1) * 128, c * TC:(c + 1) * TC],
                    )
                    tiles.append(xt_t)
                xts[c] = tiles

            emit_x_load(0)
            nc.sync.dma_start(out=wkv_sb, in_=wkv_d.ap().rearrange("(n p) m -> p n m", p=128))
            emit_x_load(1)

            cos_sb = cp.tile([128, NT, 4, 128], FP16, tag="cos")
            sin_sb = cp.tile([128, NT, 4, 128], FP16, tag="sin")
            logp_sb = cp.tile([128, NT, 1], F32, tag="logp")
            triw_sb = cp.tile([128, 512], FP16, tag="triw")
            ident_sb = cp.tile([128, 128], FP16, tag="ident")
            scal_sb = cp.tile([128, 4], F32, tag="scal")
            gn_sb = cp.tile([128, 2, 256], F32, tag="gn")
            neglam_sb = cp.tile([128, 1], F32, tag="neglam")
            magic_sb = cp.tile([128, 24], I32, tag="magic")
            nc.scalar.dma_start(out=cos_sb, in_=cos_c.ap())
            nc.scalar.dma_start(out=sin_sb, in_=sin_c.ap())
            nc.scalar.dma_start(out=logp_sb, in_=logp_c.ap())
            nc.scalar.dma_start(out=triw_sb, in_=triw_c.ap())
            nc.scalar.dma_start(out=ident_sb, in_=ident_c.ap())
            nc.scalar.dma_start(out=scal_sb, in_=scal_d.ap())
            nc.scalar.dma_start(out=gn_sb, in_=gn_d.ap())
            nc.scalar.dma_start(out=neglam_sb, in_=neglam_d.ap())
            nc.vector.memset(magic_sb[:], RSQRT_MAGIC)

            # per-token-tile persistent K^T and V(+ones)
            kT_t = [kvp.tile([128, 2, 128], FP16, tag=f"kT{i}", name=f"kT{i}") for i in range(NT)]
            vA_t = [kvp.tile([128, 258], BF16, tag=f"vA{i}", name=f"vA{i}") for i in range(NT)]
            for i in range(NT):
                nc.gpsimd.memset(vA_t[i][:, 256:258], 1.0)

            ssq_ch = {}   # [128, 4, 6] f32 per chunk
            qh_ch = {}    # list of (q_h, k_h) per chunk
            rs_ch = {}    # (rsq_h [128,4,4] fp16, rsk_h [128,4,2] fp16)
            qT_ch = {}    # [128, 4, TC] fp16 per chunk (rotating pool bufs=2)

            def proj_tile(c, ti):
                tt = c * 4 + ti
                q_ps = ps_p.tile([128, 512], F32, tag="pp")
                kv_ps = ps_p.tile([128, 512], F32, tag="pp")
                for dmi in range(NDM):
                    lhs = xts[c][dmi][:, ti * 128:(ti + 1) * 128]
                    nc.tensor.matmul(q_ps[:], lhs, wq_sb[:, dmi, :],
                                     start=(dmi == 0), stop=(dmi == NDM - 1))
                for dmi in range(NDM):
                    lhs = xts[c][dmi][:, ti * 128:(ti + 1) * 128]
                    nc.tensor.matmul(kv_ps[:], lhs, wkv_sb[:, dmi, :],
                                     start=(dmi == 0), stop=(dmi == NDM - 1))
                # drain PSUM on DVE (only engine allowed); squares via mul+reduce
                q_h = qhp.tile([128, 512], FP16, tag="qh")
                k_h = qhp.tile([128, 256], FP16, tag="kh")
                nc.vector.tensor_copy(out=q_h[:], in_=q_ps[:])
                nc.vector.tensor_copy(out=k_h[:], in_=kv_ps[:, 0:256])
                nc.vector.tensor_copy(out=vA_t[tt][:, 0:256], in_=kv_ps[:, 256:512])
                if ti == 0:
                    ssq_ch[c] = wk.tile([128, 4, 6], F32, tag="ssq", name=f"ssq{c}")
                    qh_ch[c] = []
                qh_ch[c].append((q_h, k_h))
                sqv = wk.tile([128, 4, 128], FP16, tag="sqv")
                nc.vector.tensor_mul(sqv[:], q_h[:].rearrange("p (h d) -> p h d", h=4),
                                     q_h[:].rearrange("p (h d) -> p h d", h=4))
                nc.vector.tensor_reduce(ssq_ch[c][:, ti, 0:4], sqv[:], axis=AX.X, op=ALU.add)
                skv = wk.tile([128, 2, 128], FP16, tag="skv")
                nc.vector.tensor_mul(skv[:], k_h[:].rearrange("p (h d) -> p h d", h=2),
                                     k_h[:].rearrange("p (h d) -> p h d", h=2))
                nc.vector.tensor_reduce(ssq_ch[c][:, ti, 4:6], skv[:], axis=AX.X, op=ALU.add)

            def newton_rsqrt(ms_ap, n, tag):
                # rsqrt(ms) via bit-trick seed + 2 Newton iterations (DVE only)
                sh = wk.tile([128, n], I32, tag=tag + "sh")
                nc.vector.tensor_scalar(out=sh[:], in0=ms_ap.bitcast(I32), scalar1=1,
                                        scalar2=None, op0=ALU.logical_shift_right)
                y0 = wk.tile([128, n], F32, tag=tag + "y0")
                nc.vector.tensor_tensor(out=y0[:].bitcast(I32), in0=magic_sb[:, 0:n],
                                        in1=sh[:], op=ALU.subtract)
                a = wk.tile([128, n], F32, tag=tag + "a")
                for _ in range(2):
                    nc.vector.tensor_tensor(out=a[:], in0=ms_ap, in1=y0[:], op=ALU.mult)
                    nc.vector.tensor_tensor(out=a[:], in0=a[:], in1=y0[:], op=ALU.mult)
                    nc.vector.tensor_scalar(out=a[:], in0=a[:], scalar1=-0.5, scalar2=1.5,
                                            op0=ALU.mult, op1=ALU.add)
                    nc.vector.tensor_tensor(out=y0[:], in0=y0[:], in1=a[:], op=ALU.mult)
                return y0

            def newton_chunk(c):
                ms = wk.tile([128, 24], F32, tag="ms")
                nc.vector.tensor_scalar(out=ms[:], in0=ssq_ch[c][:].rearrange("p a b -> p (a b)"),
                                        scalar1=1.0 / D, scalar2=EPS, op0=ALU.mult, op1=ALU.add)
                rs = newton_rsqrt(ms[:], 24, "nq")     # [128, 24] = [128, 4t, 6]
                rsv = rs[:].rearrange("p (t k) -> p t k", t=4)
                rsq = wk.tile([128, 4, 4], F32, tag="rsq")
                nc.vector.tensor_mul(rsq[:], rsv[:, :, 0:4],
                                     _bc_last(logp_sb[:, 4 * c:4 * c + 4, 0], 4))
                nc.vector.tensor_mul(rsq[:], rsq[:], _bc_mid(scal_sb[:], 4))
                rsq_h = wk.tile([128, 4, 4], FP16, tag="rsqh")
                nc.vector.tensor_copy(out=rsq_h[:], in_=rsq[:])
                rsk_h = wk.tile([128, 4, 2], FP16, tag="rskh")
                nc.vector.tensor_copy(out=rsk_h[:], in_=rsv[:, :, 4:6])
                rs_ch[c] = (rsq_h, rsk_h)

            def rope_tile(c, ti):
                tt = c * 4 + ti
                q_h, k_h = qh_ch[c][ti]
                rsq_h, rsk_h = rs_ch[c]
                qs = wk.tile([128, 4, 128], FP16, tag="qs")
                nc.vector.tensor_mul(qs[:], q_h[:].rearrange("p (h d) -> p h d", h=4),
                                     _bc_last(rsq_h[:, ti, :], 128))
                qc = wk.tile([128, 4, 128], FP16, tag="qc")
                nc.vector.tensor_mul(qc[:], qs[:], cos_sb[:, tt])
                tq = wk.tile([128, 4, 128], FP16, tag="tq")
                nc.vector.tensor_mul(tq[:, :, 0:64], qs[:, :, 64:128], sin_sb[:, tt, :, 0:64])
                nc.vector.tensor_mul(tq[:, :, 64:128], qs[:, :, 0:64], sin_sb[:, tt, :, 64:128])
                qr = qrp.tile([128, 4, 128], FP16, tag="qr")
                nc.vector.tensor_add(qr[:], qc[:], tq[:])
                if ti == 0:
                    qT_ch[c] = qtp.tile([128, 4, TC], FP16, tag="qtc", name=f"qtc{c}")
                nc.sync.dma_start_transpose(
                    out=qT_ch[c][:, :, ti * 128:(ti + 1) * 128],
                    in_=qr[:].rearrange("p a t -> p (a t)"))
                ks = wk.tile([128, 2, 128], FP16, tag="ks")
                nc.gpsimd.tensor_mul(ks[:], k_h[:].rearrange("p (h d) -> p h d", h=2),
                                     _bc_last(rsk_h[:, ti, :], 128))
                kc = wk.tile([128, 2, 128], FP16, tag="kc")
                nc.gpsimd.tensor_mul(kc[:], ks[:], cos_sb[:, tt, 0:2])
                tk = wk.tile([128, 2, 128], FP16, tag="tk")
                nc.gpsimd.tensor_mul(tk[:, :, 0:64], ks[:, :, 64:128], sin_sb[:, tt, 0:2, 0:64])
                nc.gpsimd.tensor_mul(tk[:, :, 64:128], ks[:, :, 0:64], sin_sb[:, tt, 0:2, 64:128])
                kr = qrp.tile([128, 2, 128], FP16, tag="kr")
                nc.gpsimd.tensor_add(kr[:], kc[:], tk[:])
                nc.scalar.dma_start_transpose(
                    out=kT_t[tt][:], in_=kr[:].rearrange("p a t -> p (a t)"))

            def qkv_chunk(c):
                for ti in range(4):
                    proj_tile(c, ti)
                newton_chunk(c)
                for ti in range(4):
                    rope_tile(c, ti)

            y1_ch = {}

            def attn_scores(c, h, s):
                pts = []
                for kt in range(4 * (c + 1)):
                    j = kt - 4 * c
                    qoff = max(j, 0) * 128
                    st = ps_s.tile([128, 512], F32, tag="st")
                    if j >= 0:
                        # preload causal mask into PSUM, accumulate scores onto it
                        nc.tensor.matmul(st[:, qoff:512], ident_sb[:],
                                         triw_sb[:, 0:512 - qoff], start=True, stop=False)
                        nc.tensor.matmul(st[:, qoff:512], kT_t[kt][:, s, :],
                                         qT_ch[c][:, 2 * s + h, qoff:512],
                                         start=False, stop=True)
                    else:
                        nc.tensor.matmul(st[:], kT_t[kt][:, s, :],
                                         qT_ch[c][:, 2 * s + h, :], start=True, stop=True)
                    pt = ptp.tile([128, 512], BF16, tag="pt")
                    nc.scalar.activation(out=pt[:, qoff:512], in_=st[:, qoff:512],
                                         func=AF.Exp, scale=ISQ)
                    pts.append(pt)
                return pts

            def attn_pv(c, h, s, pts):
                if s == 0:
                    y1_ch[(c, h)] = wk.tile([128, 4, 256], F32, tag="y1", name=f"y1_{c}_{h}")
                y1 = y1_ch[(c, h)]
                yvs = []
                s2 = None
                if s == 1:
                    s2 = wk.tile([128, 4], F32, tag="s2")
                for sq in range(4):
                    qt_g = 4 * c + sq
                    o = ps_o.tile([128, 258], F32, tag="o")
                    for kt in range(qt_g + 1):
                        nc.tensor.matmul(o[:], pts[kt][:, sq * 128:(sq + 1) * 128],
                                         vA_t[kt][:], start=(kt == 0), stop=(kt == qt_g))
                    rec = wk.tile([128, 1], F32, tag="rec")
                    nc.vector.reciprocal(rec[:], o[:, 256:257])
                    if s == 0:
                        nc.vector.tensor_scalar_mul(y1[:, sq, :], o[:, 0:256], rec[:])
                    else:
                        nc.vector.tensor_scalar_mul(rec[:], rec[:], neglam_sb[:])
                        yv = yvp.tile([128, 256], F32, tag="yv")
                        nc.vector.scalar_tensor_tensor(
                            out=yv[:], in0=o[:, 0:256], scalar=rec[:],
                            in1=y1[:, sq, :], op0=ALU.mult, op1=ALU.add)
                        sq2 = wk.tile([128, 256], F32, tag="sq2")
                        nc.vector.tensor_mul(sq2[:], yv[:], yv[:])
                        nc.vector.tensor_reduce(s2[:, sq:sq + 1], sq2[:], axis=AX.X,
                                                op=ALU.add)
                        yvs.append(yv)
                if s == 1:
                    ms2 = wk.tile([128, 4], F32, tag="ms2")
                    nc.vector.tensor_scalar(out=ms2[:], in0=s2[:], scalar1=1.0 / 256,
                                            scalar2=EPS, op0=ALU.mult, op1=ALU.add)
                    rsy = newton_rsqrt(ms2[:], 4, "ne")
                    for sq in range(4):
                        qt_g = 4 * c + sq
                        yo = yop.tile([128, 256], F32, tag="yo")
                        nc.gpsimd.tensor_mul(yo[:], yvs[sq][:], gn_sb[:, h, :])
                        nc.gpsimd.tensor_mul(yo[:], yo[:], _bc_last(rsy[:, sq:sq + 1], 256))
                        nc.gpsimd.dma_start(
                            out=y_d.ap()[qt_g * 128:(qt_g + 1) * 128, h, :], in_=yo[:])

            # ---------------- schedule ----------------
            HS = [(0, 0), (0, 1), (1, 0), (1, 1)]
            qkv_chunk(0)
            emit_x_load(2)
            qkv_chunk(1)
            ropeq = []    # pending (chunk, tile) rope emissions
            for c in (0, 1):
                if c == 1:
                    emit_x_load(3)
                cn = c + 2
                for i, (h, s) in enumerate(HS):
                    pts = attn_scores(c, h, s)
                    proj_tile(cn, i)
                    if i == 3:
                        newton_chunk(cn)
                        ropeq.extend((cn, t) for t in range(4))
                    if ropeq:
                        rc, rt = ropeq.pop(0)
                        rope_tile(rc, rt)
                    attn_pv(c, h, s, pts)
            for i, (h, s) in enumerate(HS):
                pts = attn_scores(2, h, s)
                if ropeq:
                    rc, rt = ropeq.pop(0)
                    rope_tile(rc, rt)
                attn_pv(2, h, s, pts)
            for i, (h, s) in enumerate(HS):
                pts = attn_scores(3, h, s)
                attn_pv(3, h, s, pts)
    nc.compile()
    return nc


_NC = None
_last_in_maps = None


def _get_nc():
    global _NC
    if _NC is None:
        _NC = _build()
    return _NC


def kernel(x, Wq, Wk, Wv, lambda_q1, lambda_k1, lambda_q2, lambda_k2,
           softmax_scaler, gn_weight):
    x = np.asarray(x, np.float32)
    Wq = np.asarray(Wq, np.float32)
    Wk = np.asarray(Wk, np.float32)
    Wv = np.asarray(Wv, np.float32)
    lam = float(np.exp(np.sum(np.float64(lambda_q1) * np.float64(lambda_k1)))
                - np.exp(np.sum(np.float64(lambda_q2) * np.float64(lambda_k2)))
                + LAMBDA_INIT)
    softmax_scaler = np.asarray(softmax_scaler, np.float32)
    gn_weight = np.asarray(gn_weight, np.float32)

    nc = _get_nc()
    in_maps = []
    for core in range(8):
        b, r = divmod(core, 4)
        qheads = [2 * r, 2 * r + 1, 8 + 2 * r, 8 + 2 * r + 1]
        wq_c = np.concatenate([Wq[:, hh * 128:(hh + 1) * 128] for hh in qheads], axis=1)
        wkv_c = np.concatenate([
            Wk[:, r * 128:(r + 1) * 128],
            Wk[:, (4 + r) * 128:(5 + r) * 128],
            Wv[:, r * 256:(r + 1) * 256],
        ], axis=1)
        in_maps.append({
            "xT": np.ascontiguousarray(x[b].T).astype(np.float16),
            "wq": np.ascontiguousarray(wq_c).astype(np.float16),
            "wkv": np.ascontiguousarray(wkv_c).astype(np.float16),
            "scal": np.ascontiguousarray(
                np.broadcast_to(softmax_scaler[qheads].reshape(1, 4), (128, 4))).astype(np.float32),
            "gnw": np.ascontiguousarray(
                np.broadcast_to(gn_weight[2 * r:2 * r + 2].reshape(1, 2, 256), (128, 2, 256))).astype(np.float32),
            "neglam": np.full((128, 1), -lam, np.float32),
        })
    global _last_in_maps
    _last_in_maps = in_maps
    res = run_bass_kernel_spmd(nc, in_maps, list(range(8)))
    out = np.empty((B, T, 8, 256), np.float32)
    for core in range(8):
        b, r = divmod(core, 4)
        out[b, :, 2 * r:2 * r + 2, :] = res.results[core]["y"]
    return out


# revision 17
# speedup vs baseline: 1.5946x; 1.1141x over previous
"""MixerDiffAttention Trainium2 kernel (v3).

Sharding: 8 cores = batch(2) x head-group(4).  Core (b, r) computes output
heads {2r, 2r+1} of batch b: q-heads {2r,2r+1,8+2r,8+2r+1}, k-heads {r, 4+r},
v-head r.

Design:
 - Act engine runs ONLY Exp (zero activation-table reloads).
 - fp16 q/k pipeline (host-cast inputs), bf16 pt/vA for PV (bf16 range
   needed for max-free exp).  Scores fp16 x fp16 -> f32 PSUM.
 - Transposes via batched DMA xbar: one [128,512]->[128,4,128] per tile for
   q (sync queue) and one [128,256]->[128,2,128] for k (scalar queue).
 - Causal diag mask pre-loaded into PSUM by an extra matmul in the same
   accumulation group (no vector mask work).
 - RMS rsqrt via bit-trick + Newton on DVE; sums of squares via DVE
   mul+reduce; PSUM drains on DVE; SBUF-only elementwise (rope-k, final
   gn scale) on Pool.
 - Schedule: proj(c+2) + rope tiles interleaved into attn(c) segments so
   the PE and Act never wait on bursts.
"""
import os
import numpy as np
import concourse.bass as bass
import concourse.mybir as mybir
from concourse import bacc
from concourse.tile import TileContext
from concourse.bass_utils import run_bass_kernel_spmd

B, T, DM = 2, 2048, 2048
H, KVH, D = 16, 8, 128
TC = 512                  # token chunk (= q chunk)
NT = T // 128             # 16 token tiles
NCH = T // TC             # 4 chunks
NDM = DM // 128           # 16 contraction chunks
EPS = 1e-6
ROPE_BASE = 10000.0
LAMBDA_INIT = 0.8 - 0.6 * np.exp(-0.3 * 12)
F32 = mybir.dt.float32
FP16 = mybir.dt.float16
BF16 = mybir.dt.bfloat16
I32 = mybir.dt.int32
AF = mybir.ActivationFunctionType
ALU = mybir.AluOpType
AX = mybir.AxisListType
ISQ = float(1.0 / np.sqrt(D))
MASK_NEG = -60000.0       # fp16-safe; exp(ISQ*(s+MASK_NEG)) == 0 in f32
RSQRT_MAGIC = 0x5F3759DF


def _bc_mid(a, n):
    # [128, m] AP -> [128, n(bcast), m]
    return bass.AP(tensor=a.tensor, offset=a.offset, ap=[list(a.ap[0]), [0, n], list(a.ap[1])])


def _bc_last(a, n):
    # [128, m] AP -> [128, m, n(bcast)]
    return bass.AP(tensor=a.tensor, offset=a.offset, ap=[list(a.ap[0]), list(a.ap[1]), [0, n]])


def _build():
    nc = bacc.Bacc(None, target_bir_lowering=False)

    xT = nc.dram_tensor("xT", [DM, T], FP16, kind="ExternalInput")
    wq_d = nc.dram_tensor("wq", [DM, 512], FP16, kind="ExternalInput")
    wkv_d = nc.dram_tensor("wkv", [DM, 512], FP16, kind="ExternalInput")
    scal_d = nc.dram_tensor("scal", [128, 4], F32, kind="ExternalInput")
    gn_d = nc.dram_tensor("gnw", [128, 2, 256], F32, kind="ExternalInput")
    neglam_d = nc.dram_tensor("neglam", [128, 1], F32, kind="ExternalInput")
    y_d = nc.dram_tensor("y", [T, 2, 256], F32, kind="ExternalOutput")

    # constant tables, laid out [128 partitions, NT tiles, ...] host-side
    pos = np.arange(T, dtype=np.float64)
    inv = ROPE_BASE ** (-np.arange(0, D, 2, dtype=np.float64) / D)  # (64,)
    ang = np.outer(pos, inv)                                       # (T, 64)
    cos_t = np.cos(ang).reshape(NT, 128, 64)
    sin_t = np.sin(ang).reshape(NT, 128, 64)
    cosf = np.concatenate([cos_t, cos_t], axis=2)          # (NT,128,128)
    sinf = np.concatenate([sin_t, -sin_t], axis=2)
    cosf4 = np.broadcast_to(cosf[:, :, None, :], (NT, 128, 4, 128))
    sinf4 = np.broadcast_to(sinf[:, :, None, :], (NT, 128, 4, 128))
    cosf4_h = cosf4.transpose(1, 0, 2, 3).astype(np.float16).copy()
    sinf4_h = sinf4.transpose(1, 0, 2, 3).astype(np.float16).copy()
    logp_h = np.log(np.arange(1, T + 1, dtype=np.float64)).astype(np.float32)
    logp_h = logp_h.reshape(NT, 128, 1).transpose(1, 0, 2).copy()
    pidx = np.arange(128).reshape(128, 1)
    fidx = np.arange(512).reshape(1, 512)
    triw_h = np.where((pidx > fidx) & (fidx < 128), np.float16(MASK_NEG),
                      np.float16(0.0)).astype(np.float16)       # [128, 512]
    ident_h = np.eye(128, dtype=np.float16)

    cos_c = nc.inline_tensor(cosf4_h, "cos_c")
    sin_c = nc.inline_tensor(sinf4_h, "sin_c")
    logp_c = nc.inline_tensor(logp_h, "logp_c")
    triw_c = nc.inline_tensor(triw_h, "triw_c")
    ident_c = nc.inline_tensor(ident_h, "ident_c")

    with TileContext(nc) as tc:
        with (
            tc.tile_pool(name="wp", bufs=1) as wp,
            tc.tile_pool(name="cp", bufs=1) as cp,
            tc.tile_pool(name="xp", bufs=34) as xp,
            tc.tile_pool(name="kv", bufs=1) as kvp,
            tc.tile_pool(name="qt", bufs=2) as qtp,
            tc.tile_pool(name="wk", bufs=2) as wk,
            tc.tile_pool(name="qh", bufs=6) as qhp,
            tc.tile_pool(name="qr", bufs=4) as qrp,
            tc.tile_pool(name="pt", bufs=18) as ptp,
            tc.tile_pool(name="yv", bufs=5) as yvp,
            tc.tile_pool(name="yo", bufs=4) as yop,
            tc.tile_pool(name="ps_p", bufs=2, space="PSUM") as ps_p,
            tc.tile_pool(name="ps_s", bufs=4, space="PSUM") as ps_s,
            tc.tile_pool(name="ps_o", bufs=2, space="PSUM") as ps_o,
        ):
            # ---- persistent loads (spread across queues; wq + x(0) first) ----
            wq_sb = wp.tile([128, NDM, 512], FP16, tag="wq")
            wkv_sb = wp.tile([128, NDM, 512], FP16, tag="wkv")
            nc.sync.dma_start(out=wq_sb, in_=wq_d.ap().rearrange("(n p) m -> p n m", p=128))

            xts = {}

            def emit_x_load(c):
                tiles = []
                for dmi in range(NDM):
                    xt_t = xp.tile([128, TC], FP16, tag="xt")
                    nc.scalar.dma_start(
                        out=xt_t,
                        in_=xT.ap()[dmi * 128:(dmi + 1) * 128, c * TC:(c + 1) * TC],
                    )
                    tiles.append(xt_t)
                xts[c] = tiles

            emit_x_load(0)
            nc.sync.dma_start(out=wkv_sb, in_=wkv_d.ap().rearrange("(n p) m -> p n m", p=128))
            emit_x_load(1)

            cos_sb = cp.tile([128, NT, 4, 128], FP16, tag="cos")
            sin_sb = cp.tile([128, NT, 4, 128], FP16, tag="sin")
            logp_sb = cp.tile([128, NT, 1], F32, tag="logp")
            triw_sb = cp.tile([128, 512], FP16, tag="triw")
            ident_sb = cp.tile([128, 128], FP16, tag="ident")
            scal_sb = cp.tile([128, 4], F32, tag="scal")
            gn_sb = cp.tile([128, 2, 256], F32, tag="gn")
            neglam_sb = cp.tile([128, 1], F32, tag="neglam")
            magic_sb = cp.tile([128, 24], I32, tag="magic")
            nc.scalar.dma_start(out=cos_sb, in_=cos_c.ap())
            nc.scalar.dma_start(out=sin_sb, in_=sin_c.ap())
            nc.scalar.dma_start(out=logp_sb, in_=logp_c.ap())
            nc.scalar.dma_start(out=triw_sb, in_=triw_c.ap())
            nc.scalar.dma_start(out=ident_sb, in_=ident_c.ap())
            nc.scalar.dma_start(out=scal_sb, in_=scal_d.ap())
            nc.scalar.dma_start(out=gn_sb, in_=gn_d.ap())
            nc.scalar.dma_start(out=neglam_sb, in_=neglam_d.ap())
            nc.vector.memset(magic_sb[:], RSQRT_MAGIC)

            # per-token-tile persistent K^T and V(+ones)
            kT_t = [kvp.tile([128, 2, 128], FP16, tag=f"kT{i}", name=f"kT{i}") for i in range(NT)]
            vA_t = [kvp.tile([128, 258], BF16, tag=f"vA{i}", name=f"vA{i}") for i in range(NT)]
            for i in range(NT):
                nc.gpsimd.memset(vA_t[i][:, 256:258], 1.0)

            ssq_ch = {}   # [128, 4, 6] f32 per chunk
            qh_ch = {}    # list of (q_h, k_h) per chunk
            rs_ch = {}    # (rsq_h [128,4,4] fp16, rsk_h [128,4,2] fp16)
            qT_ch = {}    # [128, 4, TC] fp16 per chunk (rotating pool bufs=2)

            def proj_tile(c, ti):
                tt = c * 4 + ti
                q_ps = ps_p.tile([128, 512], F32, tag="pp")
                kv_ps = ps_p.tile([128, 512], F32, tag="pp")
                for dmi in range(NDM):
                    lhs = xts[c][dmi][:, ti * 128:(ti + 1) * 128]
                    nc.tensor.matmul(q_ps[:], lhs, wq_sb[:, dmi, :],
                                     start=(dmi == 0), stop=(dmi == NDM - 1))
                for dmi in range(NDM):
                    lhs = xts[c][dmi][:, ti * 128:(ti + 1) * 128]
                    nc.tensor.matmul(kv_ps[:], lhs, wkv_sb[:, dmi, :],
                                     start=(dmi == 0), stop=(dmi == NDM - 1))
                # drain PSUM on DVE (only engine allowed); squares via mul+reduce
                q_h = qhp.tile([128, 512], FP16, tag="qh")
                k_h = qhp.tile([128, 256], FP16, tag="kh")
                nc.vector.tensor_copy(out=q_h[:], in_=q_ps[:])
                nc.vector.tensor_copy(out=k_h[:], in_=kv_ps[:, 0:256])
                nc.vector.tensor_copy(out=vA_t[tt][:, 0:256], in_=kv_ps[:, 256:512])
                if ti == 0:
                    ssq_ch[c] = wk.tile([128, 4, 6], F32, tag="ssq", name=f"ssq{c}")
                    qh_ch[c] = []
                qh_ch[c].append((q_h, k_h))
                sqv = wk.tile([128, 4, 128], FP16, tag="sqv")
                nc.vector.tensor_mul(sqv[:], q_h[:].rearrange("p (h d) -> p h d", h=4),
                                     q_h[:].rearrange("p (h d) -> p h d", h=4))
                nc.vector.tensor_reduce(ssq_ch[c][:, ti, 0:4], sqv[:], axis=AX.X, op=ALU.add)
                skv = wk.tile([128, 2, 128], FP16, tag="skv")
                nc.vector.tensor_mul(skv[:], k_h[:].rearrange("p (h d) -> p h d", h=2),
                                     k_h[:].rearrange("p (h d) -> p h d", h=2))
                nc.vector.tensor_reduce(ssq_ch[c][:, ti, 4:6], skv[:], axis=AX.X, op=ALU.add)

            def newton_rsqrt(ms_ap, n, tag, iters=2):
                # rsqrt(ms) via bit-trick seed + Newton iterations (DVE only)
                sh = wk.tile([128, n], I32, tag=tag + "sh")
                nc.vector.tensor_scalar(out=sh[:], in0=ms_ap.bitcast(I32), scalar1=1,
                                        scalar2=None, op0=ALU.logical_shift_right)
                y0 = wk.tile([128, n], F32, tag=tag + "y0")
                nc.vector.tensor_tensor(out=y0[:].bitcast(I32), in0=magic_sb[:, 0:n],
                                        in1=sh[:], op=ALU.subtract)
                a = wk.tile([128, n], F32, tag=tag + "a")
                for _ in range(iters):
                    nc.vector.tensor_tensor(out=a[:], in0=ms_ap, in1=y0[:], op=ALU.mult)
                    nc.vector.tensor_tensor(out=a[:], in0=a[:], in1=y0[:], op=ALU.mult)
                    nc.vector.tensor_scalar(out=a[:], in0=a[:], scalar1=-0.5, scalar2=1.5,
                                            op0=ALU.mult, op1=ALU.add)
                    nc.vector.tensor_tensor(out=y0[:], in0=y0[:], in1=a[:], op=ALU.mult)
                return y0

            def newton_chunk(c):
                ms = wk.tile([128, 24], F32, tag="ms")
                nc.vector.tensor_scalar(out=ms[:], in0=ssq_ch[c][:].rearrange("p a b -> p (a b)"),
                                        scalar1=1.0 / D, scalar2=EPS, op0=ALU.mult, op1=ALU.add)
                rs = newton_rsqrt(ms[:], 24, "nq")     # [128, 24] = [128, 4t, 6]
                rsv = rs[:].rearrange("p (t k) -> p t k", t=4)
                rsq = wk.tile([128, 4, 4], F32, tag="rsq")
                nc.vector.tensor_mul(rsq[:], rsv[:, :, 0:4],
                                     _bc_last(logp_sb[:, 4 * c:4 * c + 4, 0], 4))
                nc.vector.tensor_mul(rsq[:], rsq[:], _bc_mid(scal_sb[:], 4))
                rsq_h = wk.tile([128, 4, 4], FP16, tag="rsqh")
                nc.vector.tensor_copy(out=rsq_h[:], in_=rsq[:])
                rsk_h = wk.tile([128, 4, 2], FP16, tag="rskh")
                nc.vector.tensor_copy(out=rsk_h[:], in_=rsv[:, :, 4:6])
                rs_ch[c] = (rsq_h, rsk_h)

            def rope_tile(c, ti):
                tt = c * 4 + ti
                q_h, k_h = qh_ch[c][ti]
                rsq_h, rsk_h = rs_ch[c]
                qs = wk.tile([128, 4, 128], FP16, tag="qs")
                nc.vector.tensor_mul(qs[:], q_h[:].rearrange("p (h d) -> p h d", h=4),
                                     _bc_last(rsq_h[:, ti, :], 128))
                qc = wk.tile([128, 4, 128], FP16, tag="qc")
                nc.vector.tensor_mul(qc[:], qs[:], cos_sb[:, tt])
                tq = wk.tile([128, 4, 128], FP16, tag="tq")
                nc.vector.tensor_mul(tq[:, :, 0:64], qs[:, :, 64:128], sin_sb[:, tt, :, 0:64])
                nc.vector.tensor_mul(tq[:, :, 64:128], qs[:, :, 0:64], sin_sb[:, tt, :, 64:128])
                qr = qrp.tile([128, 4, 128], FP16, tag="qr")
                nc.vector.tensor_add(qr[:], qc[:], tq[:])
                if ti == 0:
                    qT_ch[c] = qtp.tile([128, 4, TC], FP16, tag="qtc", name=f"qtc{c}")
                nc.sync.dma_start_transpose(
                    out=qT_ch[c][:, :, ti * 128:(ti + 1) * 128],
                    in_=qr[:].rearrange("p a t -> p (a t)"))
                ks = wk.tile([128, 2, 128], FP16, tag="ks")
                nc.gpsimd.tensor_mul(ks[:], k_h[:].rearrange("p (h d) -> p h d", h=2),
                                     _bc_last(rsk_h[:, ti, :], 128))
                kc = wk.tile([128, 2, 128], FP16, tag="kc")
                nc.gpsimd.tensor_mul(kc[:], ks[:], cos_sb[:, tt, 0:2])
                tk = wk.tile([128, 2, 128], FP16, tag="tk")
                nc.gpsimd.tensor_mul(tk[:, :, 0:64], ks[:, :, 64:128], sin_sb[:, tt, 0:2, 0:64])
                nc.gpsimd.tensor_mul(tk[:, :, 64:128], ks[:, :, 0:64], sin_sb[:, tt, 0:2, 64:128])
                kr = qrp.tile([128, 2, 128], FP16, tag="kr")
                nc.gpsimd.tensor_add(kr[:], kc[:], tk[:])
                nc.sync.dma_start_transpose(
                    out=kT_t[tt][:], in_=kr[:].rearrange("p a t -> p (a t)"))

            def qkv_chunk(c):
                for ti in range(4):
                    proj_tile(c, ti)
                newton_chunk(c)
                for ti in range(4):
                    rope_tile(c, ti)

            y1_ch = {}

            def attn_scores(c, h, s):
                pts = []
                for kt in range(4 * (c + 1)):
                    j = kt - 4 * c
                    qoff = max(j, 0) * 128
                    st = ps_s.tile([128, 512], F32, tag="st")
                    if j >= 0:
                        # preload causal mask into PSUM, accumulate scores onto it
                        nc.tensor.matmul(st[:, qoff:512], ident_sb[:],
                                         triw_sb[:, 0:512 - qoff], start=True, stop=False)
                        nc.tensor.matmul(st[:, qoff:512], kT_t[kt][:, s, :],
                                         qT_ch[c][:, 2 * s + h, qoff:512],
                                         start=False, stop=True)
                    else:
                        nc.tensor.matmul(st[:], kT_t[kt][:, s, :],
                                         qT_ch[c][:, 2 * s + h, :], start=True, stop=True)
                    pt = ptp.tile([128, 512], BF16, tag="pt")
                    nc.scalar.activation(out=pt[:, qoff:512], in_=st[:, qoff:512],
                                         func=AF.Exp, scale=ISQ)
                    pts.append(pt)
                return pts

            def attn_pv(c, h, s, pts):
                if s == 0:
                    y1_ch[(c, h)] = wk.tile([128, 4, 256], F32, tag="y1", name=f"y1_{c}_{h}")
                y1 = y1_ch[(c, h)]
                yvs = []
                s2 = None
                if s == 1:
                    s2 = wk.tile([128, 4], F32, tag="s2")
                for sq in range(4):
                    qt_g = 4 * c + sq
                    o = ps_o.tile([128, 258], F32, tag="o")
                    for kt in range(qt_g + 1):
                        nc.tensor.matmul(o[:], pts[kt][:, sq * 128:(sq + 1) * 128],
                                         vA_t[kt][:], start=(kt == 0), stop=(kt == qt_g))
                    rec = wk.tile([128, 1], F32, tag="rec")
                    nc.vector.reciprocal(rec[:], o[:, 256:257])
                    if s == 0:
                        nc.vector.tensor_scalar_mul(y1[:, sq, :], o[:, 0:256], rec[:])
                    else:
                        nc.vector.tensor_scalar_mul(rec[:], rec[:], neglam_sb[:])
                        yv = yvp.tile([128, 256], F32, tag="yv")
                        nc.vector.scalar_tensor_tensor(
                            out=yv[:], in0=o[:, 0:256], scalar=rec[:],
                            in1=y1[:, sq, :], op0=ALU.mult, op1=ALU.add)
                        sq2 = wk.tile([128, 256], F32, tag="sq2")
                        nc.vector.tensor_mul(sq2[:], yv[:], yv[:])
                        nc.vector.tensor_reduce(s2[:, sq:sq + 1], sq2[:], axis=AX.X,
                                                op=ALU.add)
                        yvs.append(yv)
                if s == 1:
                    ms2 = wk.tile([128, 4], F32, tag="ms2")
                    nc.vector.tensor_scalar(out=ms2[:], in0=s2[:], scalar1=1.0 / 256,
                                            scalar2=EPS, op0=ALU.mult, op1=ALU.add)
                    rsy = newton_rsqrt(ms2[:], 4, "ne", iters=1)
                    for sq in range(4):
                        qt_g = 4 * c + sq
                        yo = yop.tile([128, 256], F32, tag="yo")
                        nc.gpsimd.tensor_mul(yo[:], yvs[sq][:], gn_sb[:, h, :])
                        nc.gpsimd.tensor_mul(yo[:], yo[:], _bc_last(rsy[:, sq:sq + 1], 256))
                        nc.gpsimd.dma_start(
                            out=y_d.ap()[qt_g * 128:(qt_g + 1) * 128, h, :], in_=yo[:])

            # ---------------- schedule ----------------
            HS = [(0, 0), (0, 1), (1, 0), (1, 1)]
            qkv_chunk(0)
            emit_x_load(2)
            qkv_chunk(1)
            ropeq = []    # pending (chunk, tile) rope emissions
            for c in (0, 1):
                if c == 1:
                    emit_x_load(3)
                cn = c + 2
                for i, (h, s) in enumerate(HS):
                    if ropeq:
                        rc, rt = ropeq.pop(0)
                        rope_tile(rc, rt)
                    pts = attn_scores(c, h, s)
                    proj_tile(cn, i)
                    if i == 3:
                        newton_chunk(cn)
                        ropeq.extend((cn, t) for t in range(4))
                    attn_pv(c, h, s, pts)
            for i, (h, s) in enumerate(HS):
                if ropeq:
                    rc, rt = ropeq.pop(0)
                    rope_tile(rc, rt)
                pts = attn_scores(2, h, s)
                attn_pv(2, h, s, pts)
            for i, (h, s) in enumerate(HS):
                pts = attn_scores(3, h, s)
                attn_pv(3, h, s, pts)
    nc.compile()
    return nc


_NC = None
_last_in_maps = None


def _get_nc():
    global _NC
    if _NC is None:
        _NC = _build()
    return _NC


def kernel(x, Wq, Wk, Wv, lambda_q1, lambda_k1, lambda_q2, lambda_k2,
           softmax_scaler, gn_weight):
    x = np.asarray(x, np.float32)
    Wq = np.asarray(Wq, np.float32)
    Wk = np.asarray(Wk, np.float32)
    Wv = np.asarray(Wv, np.float32)
    lam = float(np.exp(np.sum(np.float64(lambda_q1) * np.float64(lambda_k1)))
                - np.exp(np.sum(np.float64(lambda_q2) * np.float64(lambda_k2)))
                + LAMBDA_INIT)
    softmax_scaler = np.asarray(softmax_scaler, np.float32)
    gn_weight = np.asarray(gn_weight, np.float32)

    nc = _get_nc()
    in_maps = []
    for core in range(8):
        b, r = divmod(core, 4)
        qheads = [2 * r, 2 * r + 1, 8 + 2 * r, 8 + 2 * r + 1]
        wq_c = np.concatenate([Wq[:, hh * 128:(hh + 1) * 128] for hh in qheads], axis=1)
        wkv_c = np.concatenate([
            Wk[:, r * 128:(r + 1) * 128],
            Wk[:, (4 + r) * 128:(5 + r) * 128],
            Wv[:, r * 256:(r + 1) * 256],
        ], axis=1)
        in_maps.append({
            "xT": np.ascontiguousarray(x[b].T).astype(np.float16),
            "wq": np.ascontiguousarray(wq_c).astype(np.float16),
            "wkv": np.ascontiguousarray(wkv_c).astype(np.float16),
            "scal": np.ascontiguousarray(
                np.broadcast_to(softmax_scaler[qheads].reshape(1, 4), (128, 4))).astype(np.float32),
            "gnw": np.ascontiguousarray(
                np.broadcast_to(gn_weight[2 * r:2 * r + 2].reshape(1, 2, 256), (128, 2, 256))).astype(np.float32),
            "neglam": np.full((128, 1), -lam, np.float32),
        })
    global _last_in_maps
    _last_in_maps = in_maps
    res = run_bass_kernel_spmd(nc, in_maps, list(range(8)))
    out = np.empty((B, T, 8, 256), np.float32)
    for core in range(8):
        b, r = divmod(core, 4)
        out[b, :, 2 * r:2 * r + 2, :] = res.results[core]["y"]
    return out
